# revision 1
# baseline (speedup 1.0000x reference)
"""Batched Householder reflection: s_new[b] = s[b] - 2*(v[b]@s[b])/(v[b]@v[b]) * v[b].

Full inputs v, s: [512, 512] f32. Sharded batch-parallel across 8 NeuronCores
(64 rows per core). Per core: rows on SBUF partitions, K=512 on the free axis.
v and s shards are stacked host-side into one [2, 64, 512] DRAM tensor.

Raw Bass (no Tile, no Block): this walrus codegen allows only ONE inline
sync-wait per instruction, so all cross-engine sync is standalone wait_ge.
The load is split across the two HWDGE engines (SP loads v, ACT loads s) so
the two 128KB transfers overlap; the store is likewise split K-wise across
SP/ACT. ACT prewarms its Square activation table while the DMAs fly.
  dot = rowsum(v*s)   (DVE scalar_tensor_tensor with accum_out)
  nsq = rowsum(v*v)   (ACT Square activation with accum_out, runs in parallel)
  coef = -2*dot/nsq   (tiny per-partition ops)
  out = coef*v + s    (one fused DVE op, per-partition scalar broadcast)
"""

import numpy as np

B, K = 512, 512
N_CORES = 8
B_LOC = B // N_CORES  # 64 rows per core

_nc = None


def _build():
    import concourse.bass as bass
    from concourse import mybir

    nc = bass.Bass("TRN2", debug=False, num_devices=N_CORES, num_swdge_queues=2)
    f32 = mybir.dt.float32

    vs = nc.dram_tensor("vs", [2, B_LOC, K], f32, kind="ExternalInput").ap()
    out = nc.dram_tensor("out", [B_LOC, K], f32, kind="ExternalOutput").ap()

    vs_t = nc.alloc_sbuf_tensor("vs_t", [B_LOC, 2, K], f32).ap()
    o_t = nc.alloc_sbuf_tensor("o_t", [B_LOC, K], f32).ap()
    junk_vs = nc.alloc_sbuf_tensor("junk_vs", [B_LOC, K], f32).ap()
    junk_vv = nc.alloc_sbuf_tensor("junk_vv", [B_LOC, K], f32).ap()
    warm = nc.alloc_sbuf_tensor("warm", [B_LOC, 1], f32).ap()
    dot = nc.alloc_sbuf_tensor("dot", [B_LOC, 1], f32).ap()
    nsq = nc.alloc_sbuf_tensor("nsq", [B_LOC, 1], f32).ap()
    rcp = nc.alloc_sbuf_tensor("rcp", [B_LOC, 1], f32).ap()
    coef = nc.alloc_sbuf_tensor("coef", [B_LOC, 1], f32).ap()

    dma_in = nc.alloc_semaphore("dma_in")
    act_done = nc.alloc_semaphore("act_done")
    dve_sem = nc.alloc_semaphore("dve_sem")
    dve_done = nc.alloc_semaphore("dve_done")
    dma_out = nc.alloc_semaphore("dma_out")

    mult = mybir.AluOpType.mult
    add = mybir.AluOpType.add
    Square = mybir.ActivationFunctionType.Square

    sp, act, ve = nc.sync, nc.scalar, nc.vector
    v_t = vs_t[:, 0, :]
    s_t = vs_t[:, 1, :]
    zero64 = nc.const_aps.scalar_like(0.0, dot[:])

    # ---- loads ----
    # Each issue engine's dynamic DMA queue serializes its transfers at
    # ~28 GB/s, and each dma_start costs ~600ns of issue time on the engine.
    # So fan the 256KB input across FOUR streams: SP and ACT (one HWDGE queue
    # each) take the top row-halves, Pool (SWDGE) takes the bottom halves.
    pl = nc.gpsimd
    HB = B_LOC // 2  # 32 rows
    sp.dma_start(out=vs_t[:HB, 0, :], in_=vs[0, :HB, :]).then_inc(dma_in, 16)
    act.dma_start(out=vs_t[:HB, 1, :], in_=vs[1, :HB, :]).then_inc(dma_in, 16)
    pl.dma_start(out=vs_t[HB:, 0, :], in_=vs[0, HB:, :]).then_inc(dma_in, 16)
    pl.dma_start(out=vs_t[HB:, 1, :], in_=vs[1, HB:, :]).then_inc(dma_in, 16)

    # ACT: prewarm the Square table while the DMAs are in flight
    act.activation(out=warm[:], in_=zero64, func=Square)
    act.wait_ge(dma_in, 64)
    act.activation(out=junk_vv[:], in_=v_t, func=Square, accum_out=nsq[:]).then_inc(
        act_done, 1
    )

    # DVE chain
    ve.wait_ge(dma_in, 64)
    ve.scalar_tensor_tensor(
        out=junk_vs[:],
        in0=v_t,
        scalar=1.0,
        in1=s_t,
        op0=mult,
        op1=mult,
        accum_out=dot[:],
    ).then_inc(dve_sem, 1)
    ve.wait_ge(act_done, 1)
    ve.reciprocal(out=rcp[:], in_=nsq[:]).then_inc(dve_sem, 1)
    # DVE writes are not visible to the next DVE instruction without a
    # completion wait (in-order issue != in-order write visibility).
    ve.wait_ge(dve_sem, 2)
    ve.scalar_tensor_tensor(
        out=coef[:], in0=dot[:], scalar=-2.0, in1=rcp[:], op0=mult, op1=mult
    ).then_inc(dve_sem, 1)
    ve.wait_ge(dve_sem, 3)
    ve.scalar_tensor_tensor(
        out=o_t[:],
        in0=v_t,
        scalar=coef[:],
        in1=s_t,
        op0=mult,
        op1=add,
    ).then_inc(dve_done, 2)

    # ---- stores: three streams (SP / ACT / Pool) ----
    # ACT's store issue is measurably slower (activation-pipe drain before
    # descriptor gen), so it gets the smallest chunk.
    sp.wait_ge(dve_done, 2)
    sp.dma_start(out=out[0:24, :], in_=o_t[0:24, :]).then_inc(dma_out, 16)
    act.wait_ge(dve_done, 2)
    act.dma_start(out=out[24:42, :], in_=o_t[24:42, :]).then_inc(dma_out, 16)
    pl.wait_ge(dve_done, 2)
    pl.dma_start(out=out[42:64, :], in_=o_t[42:64, :]).then_inc(dma_out, 16)

    # SP resets semaphores for re-execution (PJRT reuses the loaded NEFF;
    # semaphores persist between executions). Sems whose waiters have
    # provably passed (everything up to dve_done) clear while the store
    # transfers drain; dve_done/dma_out clear after the final wait proves
    # Pool and ACT passed their dve_done waits too.
    sp.wait_ge(dve_done, 2)
    for sem in (dma_in, act_done, dve_sem):
        sp.sem_clear(sem)
    sp.wait_ge(dma_out, 48)
    sp.sem_clear(dve_done)
    sp.sem_clear(dma_out)

    return nc


def kernel(i=None, v=None, s=None, **_):
    global _nc
    from concourse.bass_utils import run_bass_kernel_spmd

    if _nc is None:
        _nc = _build()

    v = np.asarray(v, dtype=np.float32)
    s = np.asarray(s, dtype=np.float32)
    in_maps = [
        {
            "vs": np.ascontiguousarray(
                np.stack(
                    [v[c * B_LOC : (c + 1) * B_LOC], s[c * B_LOC : (c + 1) * B_LOC]]
                )
            )
        }
        for c in range(N_CORES)
    ]
    res = run_bass_kernel_spmd(_nc, in_maps, core_ids=list(range(N_CORES)))
    return np.concatenate([r["out"] for r in res.results], axis=0)



# revision 5
# speedup vs baseline: 1.0375x; 1.0375x over previous
"""Batched Householder reflection: s_new[b] = s[b] - 2*(v[b]@s[b])/(v[b]@v[b]) * v[b].

Full inputs v, s: [512, 512] f32. Sharded batch-parallel across 8 NeuronCores
(64 rows per core). All I/O in bf16 (rel-err gate is 2e-2; bf16 end-to-end
lands ~2e-3): halves DMA bytes and doubles DVE element rate.

Layout "shift" (default): one [128, 512] bf16 SBUF tile per core, v rows on
partitions 0-63, s rows on partitions 64-127. The single load DMA touches all
128 partitions -> all 16 SDMA engines. Compute ops address s as a
partition-offset operand (in1 base = 64) while out/v sit on partitions 0-63.

  dot = rowsum(v*s)   DVE scalar_tensor_tensor accum_out
  nsq = rowsum(v*v)   ACT Square activation accum_out (parallel with dot)
  coef = (-2*dot)/nsq one DVE STT with op1=divide (replaces rcp+mul)
  out = coef*v + s    one DVE STT, per-partition scalar broadcast

Layout "flat" (fallback): [64, 2, 512] tile, v/s interleaved on the free axis
of partitions 0-63 (baseline-proven operand addressing, 8 SDMA engines).
"""

import numpy as np

B, K = 512, 512
N_CORES = 8
B_LOC = B // N_CORES  # 64 rows per core

MODE = "flat"  # "shift" | "flat"  (shift is rejected by the BIR verifier:
# TensorScalarPtr requires equal base partitions for both SBUF inputs)

_nc = None


def _build(mode=MODE):
    import concourse.bass as bass
    from concourse import mybir

    nc = bass.Bass("TRN2", debug=False, num_devices=N_CORES, num_swdge_queues=2)
    bf16 = mybir.dt.bfloat16
    f32 = mybir.dt.float32

    if mode == "shift":
        x = nc.dram_tensor("x", [2 * B_LOC, K], bf16, kind="ExternalInput").ap()
        xt = nc.alloc_sbuf_tensor("xt", [2 * B_LOC, K], bf16).ap()
        v_t = xt[0:B_LOC, :]
        s_t = xt[B_LOC : 2 * B_LOC, :]
    else:
        x = nc.dram_tensor("x", [B_LOC, 2, K], bf16, kind="ExternalInput").ap()
        xt = nc.alloc_sbuf_tensor("xt", [B_LOC, 2, K], bf16).ap()
        v_t = xt[:, 0, :]
        s_t = xt[:, 1, :]

    out = nc.dram_tensor("out", [B_LOC, K], bf16, kind="ExternalOutput").ap()

    o_t = nc.alloc_sbuf_tensor("o_t", [B_LOC, K], bf16).ap()
    junk_vs = nc.alloc_sbuf_tensor("junk_vs", [B_LOC, K], bf16).ap()
    junk_vv = nc.alloc_sbuf_tensor("junk_vv", [B_LOC, K], bf16).ap()
    warm = nc.alloc_sbuf_tensor("warm", [B_LOC, 1], f32).ap()
    dot = nc.alloc_sbuf_tensor("dot", [B_LOC, 1], f32).ap()
    nsq = nc.alloc_sbuf_tensor("nsq", [B_LOC, 1], f32).ap()
    rcp = nc.alloc_sbuf_tensor("rcp", [B_LOC, 1], f32).ap()
    coef = nc.alloc_sbuf_tensor("coef", [B_LOC, 1], f32).ap()

    dma_in = nc.alloc_semaphore("dma_in")
    act_done = nc.alloc_semaphore("act_done")
    dve_sem = nc.alloc_semaphore("dve_sem")
    dve_done = nc.alloc_semaphore("dve_done")
    dma_out = nc.alloc_semaphore("dma_out")

    mult = mybir.AluOpType.mult
    add = mybir.AluOpType.add
    Square = mybir.ActivationFunctionType.Square

    sp, act, ve = nc.sync, nc.scalar, nc.vector
    zero64 = nc.const_aps.scalar_like(0.0, dot[:])

    # ---- load: one DMA, issued by SP (HWDGE) ----
    sp.dma_start(out=xt[:], in_=x[:]).then_inc(dma_in, 16)

    # ACT: prewarm the Square table while the DMA flies
    act.activation(out=warm[:], in_=zero64, func=Square)
    act.wait_ge(dma_in, 16)
    act.activation(out=junk_vv[:], in_=v_t, func=Square, accum_out=nsq[:]).then_inc(
        act_done, 1
    )

    # DVE chain
    ve.wait_ge(dma_in, 16)
    ve.scalar_tensor_tensor(
        out=junk_vs[:],
        in0=v_t,
        scalar=1.0,
        in1=s_t,
        op0=mult,
        op1=mult,
        accum_out=dot[:],
    ).then_inc(dve_sem, 1)
    # DVE writes are not visible to the next DVE instruction without a
    # completion wait (in-order issue != in-order write visibility).
    ve.wait_ge(act_done, 1)
    ve.reciprocal(out=rcp[:], in_=nsq[:]).then_inc(dve_sem, 1)
    ve.wait_ge(dve_sem, 2)
    ve.scalar_tensor_tensor(
        out=coef[:], in0=dot[:], scalar=-2.0, in1=rcp[:], op0=mult, op1=mult
    ).then_inc(dve_sem, 1)
    ve.wait_ge(dve_sem, 3)
    ve.scalar_tensor_tensor(
        out=o_t[:],
        in0=v_t,
        scalar=coef[:],
        in1=s_t,
        op0=mult,
        op1=add,
    ).then_inc(dve_done, 1)

    # ---- store: one DMA from SP ----
    sp.wait_ge(dve_done, 1)
    sp.dma_start(out=out[:], in_=o_t[:]).then_inc(dma_out, 16)

    # SP resets semaphores for re-execution (PJRT reuses the loaded NEFF;
    # semaphores persist between executions). dve_done>=1 proves every
    # waiter of the early sems has passed.
    for sem in (dma_in, act_done, dve_sem):
        sp.sem_clear(sem)
    sp.wait_ge(dma_out, 16)
    sp.sem_clear(dve_done)
    sp.sem_clear(dma_out)

    return nc


def _shards(v, s, mode=MODE):
    import ml_dtypes

    bf16 = ml_dtypes.bfloat16
    v = np.asarray(v, dtype=np.float32).astype(bf16)
    s = np.asarray(s, dtype=np.float32).astype(bf16)
    maps = []
    for c in range(N_CORES):
        vc = v[c * B_LOC : (c + 1) * B_LOC]
        sc = s[c * B_LOC : (c + 1) * B_LOC]
        if mode == "shift":
            xc = np.ascontiguousarray(np.concatenate([vc, sc], axis=0))
        else:
            xc = np.ascontiguousarray(np.stack([vc, sc], axis=1))
        maps.append({"x": xc})
    return maps


def kernel(i=None, v=None, s=None, **_):
    global _nc
    from concourse.bass_utils import run_bass_kernel_spmd

    if _nc is None:
        _nc = _build()

    in_maps = _shards(v, s)
    res = run_bass_kernel_spmd(_nc, in_maps, core_ids=list(range(N_CORES)))
    return np.concatenate([r["out"] for r in res.results], axis=0).astype(np.float32)


# revision 7
# speedup vs baseline: 1.1126x; 1.0725x over previous
"""Batched Householder reflection: s_new[b] = s[b] - 2*(v[b]@s[b])/(v[b]@v[b]) * v[b].

Full inputs v, s: [512, 512] f32. Sharded batch-parallel across 8 NeuronCores
(64 rows per core). All I/O in bf16 (rel-err gate is 2e-2; bf16 end-to-end
lands ~2.4e-3): halves DMA bytes. Compute speed is dtype-independent here
(STT/activation have no DVE 2x perf mode), so bf16 only buys DMA time.

Per core one [64, 2, 512] bf16 tile: row b holds v[b] | s[b] on partition b.
Dynamic-DMA queues are the bottleneck (~40-60 GB/s per queue), so the 128KB
load is fanned across all 4 queues (SP + ACT HWDGE, 2 Pool SWDGE), skewed so
the late-starting Pool queues carry less. Store is split SP/ACT.

  dot = rowsum(v*s)   DVE scalar_tensor_tensor accum_out
  nsq = rowsum(v*v)   ACT Square activation accum_out (parallel with dot)
  rcp = 1/nsq; coef = -2*dot*rcp; out = coef*v + s (DVE)
"""

import numpy as np

B, K = 512, 512
N_CORES = 8
B_LOC = B // N_CORES  # 64 rows per core

# load split row boundaries: SP / ACT (HWDGE only; SWDGE engines 7/15
# straggle by ~2.7us under descriptor-ring port contention)
LS = (0, 32, 64)

_nc = None


def _build():
    import concourse.bass as bass
    from concourse import mybir

    nc = bass.Bass("TRN2", debug=False, num_devices=N_CORES, num_swdge_queues=1)
    bf16 = mybir.dt.bfloat16
    f32 = mybir.dt.float32

    x = nc.dram_tensor("x", [B_LOC, 2, K], bf16, kind="ExternalInput").ap()
    xt = nc.alloc_sbuf_tensor("xt", [B_LOC, 2, K], bf16).ap()
    v_t = xt[:, 0, :]
    s_t = xt[:, 1, :]

    out = nc.dram_tensor("out", [B_LOC, K], bf16, kind="ExternalOutput").ap()

    o_t = nc.alloc_sbuf_tensor("o_t", [B_LOC, K], bf16).ap()
    junk_vs = nc.alloc_sbuf_tensor("junk_vs", [B_LOC, K], bf16).ap()
    junk_vv = nc.alloc_sbuf_tensor("junk_vv", [B_LOC, K], bf16).ap()
    warm = nc.alloc_sbuf_tensor("warm", [B_LOC, 1], f32).ap()
    dot = nc.alloc_sbuf_tensor("dot", [B_LOC, 1], f32).ap()
    nsq = nc.alloc_sbuf_tensor("nsq", [B_LOC, 1], f32).ap()
    rcp = nc.alloc_sbuf_tensor("rcp", [B_LOC, 1], f32).ap()
    coef = nc.alloc_sbuf_tensor("coef", [B_LOC, 1], f32).ap()

    dma_in = nc.alloc_semaphore("dma_in")
    act_done = nc.alloc_semaphore("act_done")
    dve_sem = nc.alloc_semaphore("dve_sem")
    dve_done = nc.alloc_semaphore("dve_done")
    dma_out = nc.alloc_semaphore("dma_out")

    mult = mybir.AluOpType.mult
    add = mybir.AluOpType.add
    Square = mybir.ActivationFunctionType.Square

    sp, act, ve = nc.sync, nc.scalar, nc.vector
    zero64 = nc.const_aps.scalar_like(0.0, dot[:])

    # ---- load: 2 HWDGE queues (SP, ACT) ----
    a, b, c = LS
    sp.dma_start(out=xt[a:b], in_=x[a:b]).then_inc(dma_in, 16)
    act.dma_start(out=xt[b:c], in_=x[b:c]).then_inc(dma_in, 16)

    # ACT: prewarm the Square table while the DMAs fly
    act.activation(out=warm[:], in_=zero64, func=Square)
    act.wait_ge(dma_in, 32)
    act.activation(out=junk_vv[:], in_=v_t, func=Square, accum_out=nsq[:]).then_inc(
        act_done, 1
    )

    # DVE chain
    ve.wait_ge(dma_in, 32)
    ve.scalar_tensor_tensor(
        out=junk_vs[:],
        in0=v_t,
        scalar=1.0,
        in1=s_t,
        op0=mult,
        op1=mult,
        accum_out=dot[:],
    ).then_inc(dve_sem, 1)
    # DVE writes are not visible to the next DVE instruction without a
    # completion wait (in-order issue != in-order write visibility).
    ve.wait_ge(act_done, 1)
    ve.reciprocal(out=rcp[:], in_=nsq[:]).then_inc(dve_sem, 1)
    ve.wait_ge(dve_sem, 2)
    ve.scalar_tensor_tensor(
        out=coef[:], in0=dot[:], scalar=-2.0, in1=rcp[:], op0=mult, op1=mult
    ).then_inc(dve_sem, 1)
    ve.wait_ge(dve_sem, 3)
    ve.scalar_tensor_tensor(
        out=o_t[:],
        in0=v_t,
        scalar=coef[:],
        in1=s_t,
        op0=mult,
        op1=add,
    ).then_inc(dve_done, 2)

    # ---- store: SP and ACT halves in parallel ----
    HB = B_LOC // 2
    sp.wait_ge(dve_done, 2)
    sp.dma_start(out=out[:HB], in_=o_t[:HB]).then_inc(dma_out, 16)
    act.wait_ge(dve_done, 2)
    act.dma_start(out=out[HB:], in_=o_t[HB:]).then_inc(dma_out, 16)

    # SP resets semaphores for re-execution (PJRT reuses the loaded NEFF;
    # semaphores persist between executions). dve_done>=2 proves every
    # waiter of the early sems has passed.
    for sem in (dma_in, act_done, dve_sem):
        sp.sem_clear(sem)
    sp.wait_ge(dma_out, 32)
    sp.sem_clear(dve_done)
    sp.sem_clear(dma_out)

    return nc


def _shards(v, s):
    import ml_dtypes

    bf16 = ml_dtypes.bfloat16
    v = np.asarray(v, dtype=np.float32).astype(bf16)
    s = np.asarray(s, dtype=np.float32).astype(bf16)
    maps = []
    for c in range(N_CORES):
        vc = v[c * B_LOC : (c + 1) * B_LOC]
        sc = s[c * B_LOC : (c + 1) * B_LOC]
        maps.append({"x": np.ascontiguousarray(np.stack([vc, sc], axis=1))})
    return maps


def kernel(i=None, v=None, s=None, **_):
    global _nc
    from concourse.bass_utils import run_bass_kernel_spmd

    if _nc is None:
        _nc = _build()

    in_maps = _shards(v, s)
    res = run_bass_kernel_spmd(_nc, in_maps, core_ids=list(range(N_CORES)))
    return np.concatenate([r["out"] for r in res.results], axis=0).astype(np.float32)


# revision 9
# speedup vs baseline: 1.1743x; 1.0554x over previous
"""Batched Householder reflection: s_new[b] = s[b] - 2*(v[b]@s[b])/(v[b]@v[b]) * v[b].

Full inputs v, s: [512, 512] f32. Sharded batch-parallel across 8 NeuronCores
(64 rows per core). All I/O in bf16 (rel-err gate is 2e-2; bf16 end-to-end
lands ~2.4e-3): halves DMA bytes. Compute speed is dtype-independent here
(STT/activation have no DVE 2x perf mode), so bf16 only buys DMA time.

Per core one [64, 2, 512] bf16 tile: row b holds v[b] | s[b] on partition b.
Dynamic-DMA queues are the bottleneck (~40-60 GB/s per queue), so the 128KB
load is fanned across all 4 queues (SP + ACT HWDGE, 2 Pool SWDGE), skewed so
the late-starting Pool queues carry less. Store is split SP/ACT.

  dot = rowsum(v*s)   DVE scalar_tensor_tensor accum_out
  nsq = rowsum(v*v)   ACT Square activation accum_out (parallel with dot)
  rcp = 1/nsq; coef = -2*dot*rcp; out = coef*v + s (DVE)
"""

import numpy as np

B, K = 512, 512
N_CORES = 8
B_LOC = B // N_CORES  # 64 rows per core

# load split row boundaries: SP / ACT (HWDGE only; SWDGE engines 7/15
# straggle by ~2.7us under descriptor-ring port contention)
LS = (0, 32, 64)

_nc = None


def _build():
    import concourse.bass as bass
    from concourse import mybir

    nc = bass.Bass("TRN2", debug=False, num_devices=N_CORES, num_swdge_queues=1)
    bf16 = mybir.dt.bfloat16
    f32 = mybir.dt.float32

    x = nc.dram_tensor("x", [B_LOC, 2, K], bf16, kind="ExternalInput").ap()
    xt = nc.alloc_sbuf_tensor("xt", [B_LOC, 2, K], bf16).ap()
    v_t = xt[:, 0, :]
    s_t = xt[:, 1, :]

    out = nc.dram_tensor("out", [B_LOC, K], bf16, kind="ExternalOutput").ap()

    o_t = nc.alloc_sbuf_tensor("o_t", [B_LOC, K], bf16).ap()
    junk_vs = nc.alloc_sbuf_tensor("junk_vs", [B_LOC, K], bf16).ap()
    junk_vv = nc.alloc_sbuf_tensor("junk_vv", [B_LOC, K], bf16).ap()
    warm = nc.alloc_sbuf_tensor("warm", [B_LOC, 1], f32).ap()
    dot = nc.alloc_sbuf_tensor("dot", [B_LOC, 1], f32).ap()
    nsq = nc.alloc_sbuf_tensor("nsq", [B_LOC, 1], f32).ap()
    rcp = nc.alloc_sbuf_tensor("rcp", [B_LOC, 1], f32).ap()
    coef = nc.alloc_sbuf_tensor("coef", [B_LOC, 1], f32).ap()

    dma_in = nc.alloc_semaphore("dma_in")
    act_done = nc.alloc_semaphore("act_done")
    dve_sem = nc.alloc_semaphore("dve_sem")
    dve_done = nc.alloc_semaphore("dve_done")
    act_issued = nc.alloc_semaphore("act_issued")
    dma_out = nc.alloc_semaphore("dma_out")

    mult = mybir.AluOpType.mult
    add = mybir.AluOpType.add
    Square = mybir.ActivationFunctionType.Square

    sp, act, ve = nc.sync, nc.scalar, nc.vector
    zero64 = nc.const_aps.scalar_like(0.0, dot[:])

    # ---- load: 2 HWDGE queues (SP, ACT) ----
    # dma_out carries the previous execution's (unwaited) store completions;
    # the runtime drained those rings before relaunching, so clear it here.
    sp.sem_clear(dma_out)
    a, b, c = LS
    sp.dma_start(out=xt[a:b], in_=x[a:b]).then_inc(dma_in, 16)
    act.dma_start(out=xt[b:c], in_=x[b:c]).then_inc(dma_in, 16)

    # ACT: prewarm the Square table while the DMAs fly
    act.activation(out=warm[:], in_=zero64, func=Square)
    act.wait_ge(dma_in, 32)
    act.activation(out=junk_vv[:], in_=v_t, func=Square, accum_out=nsq[:]).then_inc(
        act_done, 1
    )

    # DVE chain
    ve.wait_ge(dma_in, 32)
    ve.scalar_tensor_tensor(
        out=junk_vs[:],
        in0=v_t,
        scalar=1.0,
        in1=s_t,
        op0=mult,
        op1=mult,
        accum_out=dot[:],
    ).then_inc(dve_sem, 1)
    # DVE writes are not visible to the next DVE instruction without a
    # completion wait (in-order issue != in-order write visibility).
    ve.wait_ge(act_done, 1)
    ve.reciprocal(out=rcp[:], in_=nsq[:]).then_inc(dve_sem, 1)
    ve.wait_ge(dve_sem, 2)
    ve.scalar_tensor_tensor(
        out=coef[:], in0=dot[:], scalar=-2.0, in1=rcp[:], op0=mult, op1=mult
    ).then_inc(dve_sem, 1)
    ve.wait_ge(dve_sem, 3)
    ve.scalar_tensor_tensor(
        out=o_t[:],
        in0=v_t,
        scalar=coef[:],
        in1=s_t,
        op0=mult,
        op1=add,
    ).then_inc(dve_done, 2)

    # ---- store: SP and ACT halves in parallel, NO completion semaphore ----
    # The sequencers don't wait for the store to land: the runtime drains the
    # DMA rings at execution end, so the postamble (per-engine event clears +
    # end barrier) overlaps the store drain instead of following it.
    HB = B_LOC // 2
    sp.wait_ge(dve_done, 2)
    sp.dma_start(out=out[:HB], in_=o_t[:HB]).then_inc(dma_out, 16)
    act.wait_ge(dve_done, 2)
    act.dma_start(out=out[HB:], in_=o_t[HB:]).then_inc(dma_out, 16)
    act.sem_inc(act_issued, 1)

    # SP resets semaphores for re-execution (PJRT reuses the loaded NEFF;
    # semaphores persist between executions). dve_done>=2 proves every
    # waiter of the early sems has passed; act_issued proves ACT passed its
    # dve_done wait, so dve_done is safe to clear.
    for sem in (dma_in, act_done, dve_sem):
        sp.sem_clear(sem)
    sp.wait_ge(act_issued, 1)
    sp.sem_clear(dve_done)
    sp.sem_clear(act_issued)

    return nc


def _shards(v, s):
    import ml_dtypes

    bf16 = ml_dtypes.bfloat16
    v = np.asarray(v, dtype=np.float32).astype(bf16)
    s = np.asarray(s, dtype=np.float32).astype(bf16)
    maps = []
    for c in range(N_CORES):
        vc = v[c * B_LOC : (c + 1) * B_LOC]
        sc = s[c * B_LOC : (c + 1) * B_LOC]
        maps.append({"x": np.ascontiguousarray(np.stack([vc, sc], axis=1))})
    return maps


def kernel(i=None, v=None, s=None, **_):
    global _nc
    from concourse.bass_utils import run_bass_kernel_spmd

    if _nc is None:
        _nc = _build()

    in_maps = _shards(v, s)
    res = run_bass_kernel_spmd(_nc, in_maps, core_ids=list(range(N_CORES)))
    return np.concatenate([r["out"] for r in res.results], axis=0).astype(np.float32)


# revision 10
# speedup vs baseline: 1.4418x; 1.2278x over previous
"""Batched Householder reflection: s_new[b] = s[b] - 2*(v[b]@s[b])/(v[b]@v[b]) * v[b].

Full inputs v, s: [512, 512] f32. Sharded batch-parallel across 8 NeuronCores
(64 rows per core). All I/O in bf16 (rel-err gate is 2e-2; bf16 end-to-end
lands ~2.4e-3): halves DMA bytes. Compute speed is dtype-independent here
(STT/activation have no DVE 2x perf mode), so bf16 only buys DMA time.

Per core one [64, 2, 512] bf16 tile: row b holds v[b] | s[b] on partition b.
Dynamic-DMA queues are the bottleneck (~40-60 GB/s per queue), so the 128KB
load is fanned across all 4 queues (SP + ACT HWDGE, 2 Pool SWDGE), skewed so
the late-starting Pool queues carry less. Store is split SP/ACT.

  dot = rowsum(v*s)   DVE scalar_tensor_tensor accum_out
  nsq = rowsum(v*v)   ACT Square activation accum_out (parallel with dot)
  rcp = 1/nsq; coef = -2*dot*rcp; out = coef*v + s (DVE)
"""

import numpy as np

B, K = 512, 512
N_CORES = 8
B_LOC = B // N_CORES  # 64 rows per core

# load split row boundaries: SP / ACT (HWDGE only; SWDGE engines 7/15
# straggle by ~2.7us under descriptor-ring port contention)
LS = (0, 32, 64)

_nc = None


def _build():
    import concourse.bass as bass
    from concourse import mybir

    nc = bass.Bass("TRN2", debug=False, num_devices=N_CORES, num_swdge_queues=1)
    bf16 = mybir.dt.bfloat16
    f32 = mybir.dt.float32

    # Preamble surgery: drop the framework's const-tile MEMSETs (unused here)
    # and the post-init all-engine barrier. The runtime's own engine barrier
    # right before `main` already orders everything the kernel needs, so user
    # DMAs can issue ~1.1us earlier.
    bb = nc.main_func.blocks[0]
    keep = [
        i
        for i in bb.instructions
        if type(i).__name__ not in ("InstMemset", "InstDrain", "InstEventSemaphore")
    ]
    del bb.instructions[:]
    bb.instructions.extend(keep)

    x = nc.dram_tensor("x", [B_LOC, 2, K], bf16, kind="ExternalInput").ap()
    xt = nc.alloc_sbuf_tensor("xt", [B_LOC, 2, K], bf16).ap()
    v_t = xt[:, 0, :]
    s_t = xt[:, 1, :]

    out = nc.dram_tensor("out", [B_LOC, K], bf16, kind="ExternalOutput").ap()

    o_t = nc.alloc_sbuf_tensor("o_t", [B_LOC, K], bf16).ap()
    junk_vs = nc.alloc_sbuf_tensor("junk_vs", [B_LOC, K], bf16).ap()
    junk_vv = nc.alloc_sbuf_tensor("junk_vv", [B_LOC, K], bf16).ap()
    warm = nc.alloc_sbuf_tensor("warm", [B_LOC, 1], f32).ap()
    dot = nc.alloc_sbuf_tensor("dot", [B_LOC, 1], f32).ap()
    nsq = nc.alloc_sbuf_tensor("nsq", [B_LOC, 1], f32).ap()
    rcp = nc.alloc_sbuf_tensor("rcp", [B_LOC, 1], f32).ap()
    coef = nc.alloc_sbuf_tensor("coef", [B_LOC, 1], f32).ap()

    dma_in = nc.alloc_semaphore("dma_in")
    act_done = nc.alloc_semaphore("act_done")
    dve_sem = nc.alloc_semaphore("dve_sem")
    dve_done = nc.alloc_semaphore("dve_done")
    act_issued = nc.alloc_semaphore("act_issued")
    dma_out = nc.alloc_semaphore("dma_out")

    mult = mybir.AluOpType.mult
    add = mybir.AluOpType.add
    Square = mybir.ActivationFunctionType.Square

    sp, act, ve = nc.sync, nc.scalar, nc.vector

    # ---- load: 2 HWDGE queues (SP, ACT) ----
    # dma_out carries the previous execution's (unwaited) store completions;
    # the runtime drained those rings before relaunching, so clear it here.
    sp.sem_clear(dma_out)
    a, b, c = LS
    sp.dma_start(out=xt[a:b], in_=x[a:b]).then_inc(dma_in, 16)
    act.dma_start(out=xt[b:c], in_=x[b:c]).then_inc(dma_in, 16)

    # ACT: prewarm the Square table while the DMAs fly (input is garbage)
    act.activation(out=warm[:], in_=dot[:], func=Square)
    act.wait_ge(dma_in, 32)
    act.activation(out=junk_vv[:], in_=v_t, func=Square, accum_out=nsq[:]).then_inc(
        act_done, 1
    )

    # DVE chain
    ve.wait_ge(dma_in, 32)
    ve.scalar_tensor_tensor(
        out=junk_vs[:],
        in0=v_t,
        scalar=1.0,
        in1=s_t,
        op0=mult,
        op1=mult,
        accum_out=dot[:],
    ).then_inc(dve_sem, 1)
    # DVE writes are not visible to the next DVE instruction without a
    # completion wait (in-order issue != in-order write visibility).
    ve.wait_ge(act_done, 1)
    ve.reciprocal(out=rcp[:], in_=nsq[:]).then_inc(dve_sem, 1)
    ve.wait_ge(dve_sem, 2)
    ve.scalar_tensor_tensor(
        out=coef[:], in0=dot[:], scalar=-2.0, in1=rcp[:], op0=mult, op1=mult
    ).then_inc(dve_sem, 1)
    # Final op split by K-halves: SP's store of the first half issues while
    # DVE computes the second half.
    HK = K // 2
    ve.wait_ge(dve_sem, 3)
    ve.scalar_tensor_tensor(
        out=o_t[:, :HK],
        in0=v_t[:, :HK],
        scalar=coef[:],
        in1=s_t[:, :HK],
        op0=mult,
        op1=add,
    ).then_inc(dve_done, 1)
    ve.scalar_tensor_tensor(
        out=o_t[:, HK:],
        in0=v_t[:, HK:],
        scalar=coef[:],
        in1=s_t[:, HK:],
        op0=mult,
        op1=add,
    ).then_inc(dve_done, 1)

    # ---- store: K-halves on SP and ACT; sequencers do NOT wait for the
    # store to land (the runtime drains DMA rings at execution end), so the
    # postamble overlaps the store drain.
    sp.wait_ge(dve_done, 1)
    sp.dma_start(out=out[:, :HK], in_=o_t[:, :HK]).then_inc(dma_out, 16)
    act.wait_ge(dve_done, 2)
    act.dma_start(out=out[:, HK:], in_=o_t[:, HK:]).then_inc(dma_out, 16)
    act.sem_inc(act_issued, 1)

    # SP resets semaphores for re-execution (PJRT reuses the loaded NEFF;
    # semaphores persist between executions). SP's dve_done wait proves DVE
    # (and hence ACT's square) passed the early sems; act_issued proves ACT
    # passed its dve_done wait, so dve_done is safe to clear.
    for sem in (dma_in, act_done, dve_sem):
        sp.sem_clear(sem)
    sp.wait_ge(act_issued, 1)
    sp.sem_clear(dve_done)
    sp.sem_clear(act_issued)

    return nc


def _shards(v, s):
    import ml_dtypes

    bf16 = ml_dtypes.bfloat16
    v = np.asarray(v, dtype=np.float32).astype(bf16)
    s = np.asarray(s, dtype=np.float32).astype(bf16)
    maps = []
    for c in range(N_CORES):
        vc = v[c * B_LOC : (c + 1) * B_LOC]
        sc = s[c * B_LOC : (c + 1) * B_LOC]
        maps.append({"x": np.ascontiguousarray(np.stack([vc, sc], axis=1))})
    return maps


def kernel(i=None, v=None, s=None, **_):
    global _nc
    from concourse.bass_utils import run_bass_kernel_spmd

    if _nc is None:
        _nc = _build()

    in_maps = _shards(v, s)
    res = run_bass_kernel_spmd(_nc, in_maps, core_ids=list(range(N_CORES)))
    return np.concatenate([r["out"] for r in res.results], axis=0).astype(np.float32)
